# revision 32
# baseline (speedup 1.0000x reference)
"""CriticRNN (embed -> GRU scan -> critic head) on 8 trn2 NeuronCores.

Sharding: data-parallel over batch B=128 -> 16 per core; weights replicated;
the T=128 GRU scan stays local per core (no collectives).

Layout: everything on-chip is TRANSPOSED (feature dim on partitions, batch on
the free dim) so GRU gate math runs on 128 partitions, biases are
per-partition scalars, and no on-chip transposes are ever needed. Host numpy
does all input/output transposes.

Phases per core (PE matmuls in bf16, fp32 PSUM accumulate):
  E: embT[H, T*16]  = relu(W_emb.T @ wsT + b_emb)
  X: xiT[3H, T*16]  = Wi.T @ embT + bi          (kept in SBUF)
  scan (t=0..127):  hhT = Wh.T @ h_effT, gates grouped per 128-row h-chunk
  C: critT = relu(W1.T @ yT + b1); value = W2.T @ critT + b2
"""

import sys

for _p in ("/opt/trn_rl_repo",):
    if _p not in sys.path:
        sys.path.append(_p)

import numpy as np
import ml_dtypes

T, B, OBS, H = 128, 128, 512, 1024
H3 = 3 * H
NCORES = 8
BL = B // NCORES  # batch per core

BF16 = ml_dtypes.bfloat16

_CACHE = {}


def _build_module(n_steps):
    import concourse.bacc as bacc
    import concourse.tile as tile
    import concourse.mybir as mybir

    dt = mybir.dt
    AF = mybir.ActivationFunctionType
    Alu = mybir.AluOpType

    TB = n_steps * BL
    NK_O = OBS // 128   # 4   k-chunks of OBS
    NK_H = H // 128     # 8   k-chunks of H
    NM_H = H // 128     # 8   m-chunks of H
    NM_3H = H3 // 128   # 24  m-chunks of 3H
    NCH = max(TB // 512, 1)   # moving n-chunks over time*batch
    CH = min(TB, 512)

    nc = bacc.Bacc("TRN2", target_bir_lowering=False, debug=False,
                   num_devices=NCORES)

    ei, eo = "ExternalInput", "ExternalOutput"
    wsT_d = nc.dram_tensor("wsT", [OBS, TB], dt.bfloat16, kind=ei).ap()
    hid_d = nc.dram_tensor("hid", [128, NK_H * BL], dt.bfloat16, kind=ei).ap()
    keep_d = nc.dram_tensor("keep", [128, n_steps * H // 8], dt.bfloat16, kind=ei).ap()
    wemb_d = nc.dram_tensor("wemb", [OBS, H], dt.bfloat16, kind=ei).ap()
    wi_d = nc.dram_tensor("wi", [H, H3], dt.bfloat16, kind=ei).ap()
    wh_d = nc.dram_tensor("wh", [H, H3], dt.bfloat16, kind=ei).ap()
    w1_d = nc.dram_tensor("w1", [H, H], dt.bfloat16, kind=ei).ap()
    w2_d = nc.dram_tensor("w2", [128, NK_H], dt.bfloat16, kind=ei).ap()
    bemb_d = nc.dram_tensor("bemb", [128, NM_H], dt.float32, kind=ei).ap()
    bi_d = nc.dram_tensor("bi", [128, NM_3H], dt.float32, kind=ei).ap()
    bhn_d = nc.dram_tensor("bhn", [128, H // 8], dt.bfloat16, kind=ei).ap()
    b1_d = nc.dram_tensor("b1", [128, NM_H], dt.float32, kind=ei).ap()
    b2_d = nc.dram_tensor("b2", [1, 1], dt.float32, kind=ei).ap()
    ident_d = nc.dram_tensor("ident", [128, 128], dt.bfloat16, kind=ei).ap()
    houtT_d = nc.dram_tensor("houtT", [H, BL], dt.float32, kind=eo).ap()
    val_d = nc.dram_tensor("val", [1, TB], dt.float32, kind=eo).ap()

    ts = lambda i, s: slice(i * s, (i + 1) * s)

    with tile.TileContext(nc) as tc:
        with tc.tile_pool(name="const", bufs=1) as cp, \
             tc.tile_pool(name="xi", bufs=1, space="DRAM") as xip, \
             tc.tile_pool(name="yT", bufs=1) as yp:

            # small constants resident the whole time
            keep_sb = cp.tile([128, n_steps * NM_H * BL], dt.bfloat16, tag="keep", name="keep")
            nc.sync.dma_start(keep_sb[:], keep_d[:])
            hid_sb = cp.tile([128, NK_H * BL], dt.bfloat16, tag="hid", name="hid")
            nc.sync.dma_start(hid_sb[:], hid_d[:])
            bi_sb = cp.tile([128, NM_3H], dt.float32, tag="bi", name="bi")
            nc.sync.dma_start(bi_sb[:], bi_d[:])
            bhn_sb = cp.tile([128, NM_H * BL], dt.bfloat16, tag="bhn", name="bhn")
            nc.sync.dma_start(bhn_sb[:], bhn_d[:])
            bemb_sb = cp.tile([128, NM_H], dt.float32, tag="bemb", name="bemb")
            nc.sync.dma_start(bemb_sb[:], bemb_d[:])
            b1_sb = cp.tile([128, NM_H], dt.float32, tag="b1", name="b1")
            nc.sync.dma_start(b1_sb[:], b1_d[:])
            b2_sb = cp.tile([1, 1], dt.float32, tag="b2", name="b2")
            nc.sync.dma_start(b2_sb[:], b2_d[:])

            xi_dram = xip.tile([NM_3H, 128, TB], dt.bfloat16, tag="xid",
                               name="xid")

            # ---------------- phase E + X ----------------
            with tc.tile_pool(name="embp", bufs=1) as ep:
                embT = [ep.tile([128, TB], dt.bfloat16, tag=f"emb{m}", name=f"emb{m}")
                        for m in range(NM_H)]
                with tc.tile_pool(name="wsp", bufs=1) as wsp, \
                     tc.tile_pool(name="wembp", bufs=1) as wep, \
                     tc.tile_pool(name="psE", bufs=4, space="PSUM") as psE:
                    wsT_sb = [wsp.tile([128, TB], dt.bfloat16, tag=f"ws{k}", name=f"ws{k}")
                              for k in range(NK_O)]
                    for k in range(NK_O):
                        nc.sync.dma_start(wsT_sb[k][:], wsT_d[ts(k, 128), :])
                    wemb_sb = [wep.tile([128, H], dt.bfloat16, tag=f"we{k}", name=f"we{k}")
                               for k in range(NK_O)]
                    for k in range(NK_O):
                        nc.sync.dma_start(wemb_sb[k][:], wemb_d[ts(k, 128), :])
                    for m in range(NM_H):
                        for n in range(NCH):
                            ps = psE.tile([128, CH], dt.float32, tag="psE", name="psE")
                            for k in range(NK_O):
                                nc.tensor.matmul(
                                    ps[:], wemb_sb[k][:, ts(m, 128)],
                                    wsT_sb[k][:, ts(n, CH)],
                                    start=(k == 0), stop=(k == NK_O - 1))
                            nc.scalar.activation(
                                embT[m][:, ts(n, CH)], ps[:], AF.Relu,
                                bias=bemb_sb[:, ts(m, 1)])

                with tc.tile_pool(name="wip", bufs=1) as wip, \
                     tc.tile_pool(name="xstg", bufs=4) as xsg, \
                     tc.tile_pool(name="psX", bufs=4, space="PSUM") as psX:
                    wi_sb = [wip.tile([128, H3], dt.bfloat16, tag=f"wi{k}", name=f"wi{k}")
                             for k in range(NK_H)]
                    for k in range(NK_H):
                        nc.sync.dma_start(wi_sb[k][:], wi_d[ts(k, 128), :])
                    for m in range(NM_3H):
                        for n in range(NCH):
                            ps = psX.tile([128, CH], dt.float32, tag="psX", name="psX")
                            for k in range(NK_H):
                                nc.tensor.matmul(
                                    ps[:], wi_sb[k][:, ts(m, 128)],
                                    embT[k][:, ts(n, CH)],
                                    start=(k == 0), stop=(k == NK_H - 1))
                            stg = xsg.tile([128, CH], dt.bfloat16, tag="xstg",
                                           name="xstg")
                            nc.vector.tensor_scalar_add(
                                stg[:], ps[:], bi_sb[:, ts(m, 1)])
                            nc.sync.dma_start(
                                xi_dram[m, :, ts(n, CH)], stg[:])

            # ---------------- GRU scan (+ interleaved critic) ----------
            # y_all: block j (h-chunk j) occupies cols [j*TB, (j+1)*TB);
            # step t of block j at cols j*TB + t*BL.
            y_all = yp.tile([128, NM_H * TB], dt.bfloat16, tag="yall",
                            name="yall")
            yv = y_all.rearrange("p (j c) -> p j c", j=NM_H)
            hout_all = yp.tile([128, NM_H * BL], dt.float32, tag="hout",
                               name="hout")
            NJH = NM_H // 2          # 4 h-chunks per half
            HW = NJH * BL            # 64 cols per half
            GW = NM_H * BL           # 128 cols per gate block in xts
            with tc.tile_pool(name="whp", bufs=1) as whp, \
                 tc.tile_pool(name="xtp", bufs=4) as xtp, \
                 tc.tile_pool(name="heffp", bufs=3) as hp, \
                 tc.tile_pool(name="gp", bufs=6) as gp, \
                 tc.tile_pool(name="w1p", bufs=1) as w1p, \
                 tc.tile_pool(name="crit", bufs=1) as crp, \
                 tc.tile_pool(name="valp", bufs=1) as vp, \
                 tc.tile_pool(name="psS", bufs=1, space="PSUM") as psS, \
                 tc.tile_pool(name="psC", bufs=1, space="PSUM") as psC, \
                 tc.tile_pool(name="psV", bufs=1, space="PSUM") as psV:
                wh_sb = [whp.tile([128, H3], dt.bfloat16, tag=f"wh{k}", name=f"wh{k}")
                         for k in range(NK_H)]
                for k in range(NK_H):
                    nc.sync.dma_start(wh_sb[k][:], wh_d[ts(k, 128), :])
                w1_sb = [w1p.tile([128, H], dt.bfloat16, tag=f"w1{k}", name=f"w1{k}")
                         for k in range(NK_H)]
                for k in range(NK_H):
                    nc.sync.dma_start(w1_sb[k][:], w1_d[ts(k, 128), :])
                w2_sb = w1p.tile([128, NK_H], dt.bfloat16, tag="w2", name="w2")
                nc.sync.dma_start(w2_sb[:], w2_d[:])
                critT = [crp.tile([128, TB], dt.bfloat16, tag=f"cr{m}", name=f"cr{m}")
                         for m in range(NM_H)]

                # h_eff for step 0 from the initial hidden state, split in
                # two half-tiles (h-chunks 0-3 / 4-7).
                heff = [hp.tile([128, HW], dt.bfloat16, tag=f"heff{h}",
                                name=f"heff{h}") for h in range(2)]
                for h in range(2):
                    nc.vector.tensor_mul(heff[h][:], hid_sb[:, ts(h, HW)],
                                         keep_sb[:, ts(h, HW)])

                for t in range(n_steps):
                    # prefetch this step's input-gate projections [3H, BL]
                    xts = xtp.tile([128, NM_3H * BL], dt.bfloat16, tag="xts",
                                   name="xts")
                    nc.sync.dma_start(
                        xts.rearrange("p (m c) -> p m c", c=BL),
                        xi_dram[:, :, ts(t, BL)].rearrange("m p c -> p m c"))
                    heff_nx = None
                    if t + 1 < n_steps:
                        heff_nx = [hp.tile([128, HW], dt.bfloat16,
                                           tag=f"heff{h}", name=f"heff{h}")
                                   for h in range(2)]
                    # One PSUM bank per half holds the A (k<4, reads
                    # heff[0]) and B (k>=4, heff[1]) partial sums side by
                    # side; every 16-col region is one contiguous
                    # accumulation group; gates sum A+B on the DVE.
                    # Layout: A: r 0:HW | z HW:2HW | n 2HW:3HW ; B: +3*HW.
                    psg = [psS.tile([128, 3 * HW], dt.float32, bufs=2,
                                    tag=f"ps{h}", name=f"ps{h}")
                           for h in range(2)]
                    for h in range(2):
                        for g in range(3):
                            for j in range(NJH):
                                c0 = g * H + (h * NJH + j) * 128
                                out = psg[h][:, ts(g * NJH + j, BL)]
                                for k in range(NK_H):
                                    rhs = (heff[0][:, ts(k, BL)] if k < 4
                                           else heff[1][:, ts(k - 4, BL)])
                                    nc.tensor.matmul(
                                        out, wh_sb[k][:, c0:c0 + 128], rhs,
                                        start=(k == 0), stop=(k == NK_H - 1))
                    for h in range(2):
                        hs = slice(h * HW, (h + 1) * HW)
                        xnh = xts[:, 2 * GW + h * HW:2 * GW + (h + 1) * HW]
                        # x_rz: [128, 2, HW] strided view of the r and z
                        # blocks of xts for this half.
                        xg = xts.rearrange("p (g c) -> p g c", c=GW)
                        x_rz = xg[:, 0:2, h * HW:(h + 1) * HW]
                        arz = gp.tile([128, 2 * HW], dt.float32, tag="arz",
                                      name="arz")
                        nc.vector.tensor_add(
                            arz.rearrange("p (g c) -> p g c", c=HW),
                            psg[h][:, 0:2 * HW].rearrange(
                                "p (g c) -> p g c", c=HW),
                            x_rz)
                        rz = gp.tile([128, 2 * HW], dt.float32, tag="rz",
                                     name="rz")
                        nc.scalar.activation(rz[:], arz[:], AF.Sigmoid)
                        hnb = gp.tile([128, HW], dt.float32, tag="hnb",
                                      name="hnb")
                        nc.vector.tensor_add(hnb[:], psg[h][:, 2 * HW:3 * HW],
                                             bhn_sb[:, hs])
                        rn = gp.tile([128, HW], dt.float32, tag="rn", name="rn")
                        nc.vector.tensor_mul(rn[:], rz[:, 0:HW], hnb[:])
                        an = gp.tile([128, HW], dt.float32, tag="an", name="an")
                        nc.vector.tensor_add(an[:], rn[:], xnh)
                        n_ = gp.tile([128, HW], dt.float32, tag="n", name="n")
                        nc.scalar.activation(n_[:], an[:], AF.Tanh)
                        # h_new = n + z*(h_eff - n)
                        d = gp.tile([128, HW], dt.float32, tag="d", name="d")
                        nc.vector.tensor_sub(d[:], heff[h][:], n_[:])
                        zd = gp.tile([128, HW], dt.float32, tag="zd", name="zd")
                        nc.vector.tensor_mul(zd[:], rz[:, HW:2 * HW], d[:])
                        tmp = gp.tile([128, HW], dt.float32, tag="tmp",
                                      name="tmp")
                        nc.vector.tensor_add(tmp[:], n_[:], zd[:])
                        if heff_nx is not None:
                            kb = (t + 1) * GW + h * HW
                            nc.vector.tensor_mul(heff_nx[h][:], tmp[:],
                                                 keep_sb[:, kb:kb + HW])
                        yslc = yv[:, h * NJH:(h + 1) * NJH, ts(t, BL)]
                        nc.vector.tensor_add(
                            yslc, n_.rearrange("p (j c) -> p j c", c=BL),
                            zd.rearrange("p (j c) -> p j c", c=BL))
                        if t == n_steps - 1:
                            nc.vector.tensor_copy(hout_all[:, hs], tmp[:])
                    heff = heff_nx

                nc.sync.dma_start(
                    houtT_d.rearrange("(j p) c -> p j c", p=128),
                    hout_all.rearrange("p (j c) -> p j c", c=BL))

                # ---------------- critic + value head ----------------
                for m in range(NM_H):
                    for n in range(NCH):
                        psc = psC.tile([128, CH], dt.float32, tag="psC",
                                       name="psC")
                        for k in range(NK_H):
                            nc.tensor.matmul(
                                psc[:], w1_sb[k][:, ts(m, 128)],
                                y_all[:, k * TB + n * CH:
                                      k * TB + (n + 1) * CH],
                                start=(k == 0), stop=(k == NK_H - 1))
                        nc.scalar.activation(
                            critT[m][:, ts(n, CH)], psc[:], AF.Relu,
                            bias=b1_sb[:, ts(m, 1)])
                val_sb = vp.tile([1, TB], dt.float32, tag="val", name="val")
                for n in range(NCH):
                    psv = psV.tile([1, CH], dt.float32, tag="psV", name="psV")
                    for k in range(NK_H):
                        nc.tensor.matmul(
                            psv[0:1, :], w2_sb[:, ts(k, 1)],
                            critT[k][:, ts(n, CH)],
                            start=(k == 0), stop=(k == NK_H - 1))
                    nc.vector.tensor_scalar_add(
                        val_sb[0:1, ts(n, CH)], psv[0:1, :], b2_sb[0:1, 0:1])
                nc.sync.dma_start(val_d[0:1, :], val_sb[0:1, :])

    nc.compile()
    return nc


def _get_module(n_steps):
    if n_steps not in _CACHE:
        _CACHE[n_steps] = _build_module(n_steps)
    return _CACHE[n_steps]


def _host_prep(hidden, world_state, dones, W_emb, b_emb, Wi, bi, Wh, bhn,
               W1, b1, W2, b2, n_steps):
    f32 = np.float32
    nk_h = H // 128
    shared = {
        "wemb": np.ascontiguousarray(W_emb, dtype=f32).astype(BF16),
        "wi": np.ascontiguousarray(Wi, dtype=f32).astype(BF16),
        "wh": np.ascontiguousarray(Wh, dtype=f32).astype(BF16),
        "w1": np.ascontiguousarray(W1, dtype=f32).astype(BF16),
        # W2 [H,1] -> [128, 8] with col k = W2[k*128:(k+1)*128, 0]
        "w2": np.asarray(W2, dtype=f32).reshape(nk_h, 128).T.copy().astype(BF16),
        "bemb": np.asarray(b_emb, dtype=f32).reshape(nk_h, 128).T.copy(),
        "bi": np.asarray(bi, dtype=f32).reshape(3 * nk_h, 128).T.copy(),
        # bhn expanded: col j*BL+b = bhn[j*128+p]
        "bhn": np.repeat(np.asarray(bhn, dtype=f32).reshape(nk_h, 128).T,
                         BL, axis=1).astype(BF16),
        "ident": np.eye(128, dtype=f32).astype(BF16),
        "b1": np.asarray(b1, dtype=f32).reshape(nk_h, 128).T.copy(),
        "b2": np.asarray(b2, dtype=f32).reshape(1, 1).copy(),
    }
    ws = np.asarray(world_state, dtype=f32)[:n_steps]
    dn = np.asarray(dones)[:n_steps]
    hid = np.asarray(hidden, dtype=f32)
    keep_full = (~dn).astype(f32)  # [T, B]
    in_maps = []
    for c in range(NCORES):
        bsl = slice(c * BL, (c + 1) * BL)
        # [T, BL, OBS] -> [OBS, T*BL]
        wsT = np.transpose(ws[:, bsl, :], (2, 0, 1)).reshape(OBS, n_steps * BL)
        # hidden [BL, H] -> [128, nk_h*BL]: col block j = hidden[:, j*128:+128].T
        hidT = np.transpose(hid[bsl].reshape(BL, nk_h, 128), (2, 1, 0)) \
                 .reshape(128, nk_h * BL)
        # keep8: col t*128 + j*16 + b = keep[t, b]  (j = h-chunk, 8 copies)
        k8 = np.broadcast_to(keep_full[:, bsl][:, None, :],
                             (n_steps, H // 128, BL))
        keep = np.broadcast_to(k8.reshape(1, n_steps * 128),
                               (128, n_steps * 128))
        m = dict(shared)
        m["wsT"] = np.ascontiguousarray(wsT).astype(BF16)
        m["hid"] = np.ascontiguousarray(hidT).astype(BF16)
        m["keep"] = np.ascontiguousarray(keep).astype(BF16)
        in_maps.append(m)
    return in_maps


def kernel(hidden, world_state, dones, W_emb, b_emb, Wi, bi, Wh, bhn,
           W1, b1, W2, b2, _n_steps=T, _results_out=None):
    from concourse.bass_utils import run_bass_kernel_spmd

    n_steps = _n_steps
    nc = _get_module(n_steps)
    in_maps = _host_prep(hidden, world_state, dones, W_emb, b_emb, Wi, bi,
                         Wh, bhn, W1, b1, W2, b2, n_steps)
    res = run_bass_kernel_spmd(nc, in_maps, list(range(NCORES)))
    if _results_out is not None:
        _results_out.append(res)
    h_out = np.empty((B, H), np.float32)
    value = np.empty((n_steps, B), np.float32)
    for c in range(NCORES):
        h_out[c * BL:(c + 1) * BL, :] = res.results[c]["houtT"].T
        value[:, c * BL:(c + 1) * BL] = \
            res.results[c]["val"].reshape(n_steps, BL)
    return (h_out, value)
